# revision 24
# baseline (speedup 1.0000x reference)
"""Trainium2 Bass kernel for nn_BICEPNeuralLayer.

Math: the reference module (Euler-Maruyama SDE scan -> Conv1d over time ->
time-mean -> linear projection) is LINEAR in the noise tensor, so the whole
pipeline collapses algebraically:

  paths[t] = c_b * sum_s retain^(t-s) eps_s          (c_b = feedback_b*sqrt(dt))
  mean_t(conv(paths)) folds to per-timestep weights on eps:
     out[b] = c_b/NS * (Tsum @ A[b] - T0 @ L[b] - T2 @ F[b]) + bias
  A[b,i] = sum_s gA[s] noise[b,s,i],   gA[s] = (1-retain^(NS-s))/(1-retain)
  L[b,i] = sum_s gL[s] noise[b,s,i],   gL[s] = retain^(NS-1-s)
  F[b,i] = noise[b,0,i]
  Tsum = out_w @ (W0+W1+W2), T0 = out_w @ W0, T2 = out_w @ W2  (Wk = conv_w[:,:,k])
  bias  = out_w @ conv_b + out_b

Device work per core (pure data parallel over batch, 32 samples/core):

  noise rides HBM as fp8 e3m4, quantized on the host with FIRST-ORDER ERROR
  FEEDBACK along the time axis s (q[s] = Q(n[s]+carry)). All three time
  functionals (gA, gL, delta_0) have smooth or tiny weight profiles along s,
  so the noise-shaped quantization error cancels ~70x in the sums: end-to-end
  rel err ~6e-4 (vs 1.4e-2 for plain fp8) at HALF the fp16 HBM traffic.

  stage 1: per (sample, feature-chunk): one matmul lhsT=noise[128s x <=128i]
           (fp8 stationary), rhs=g3[128s x 3] in exact fp16 (mixed-dtype
           matmul) -> psum[i, (b,v)].
  convert: DVE copies psum -> V tiles [128i, (v,b)] fp16; ACT restages the
           small L/F functionals to fp8 (L prescaled 1/8).
  stage 2: 24 accumulating matmuls lhsT=V[128i x 32b], rhs=mcat[128i x 512j]
           fp16 (sA refolded into the Tsum rows) -> psum[32b, 512j], then one
           tensor_scalar multiply by per-sample c_b (+ bias path if nonzero)
           and DMA out.

  DMA: noise host-pre-transposed to [s, b, i] so each descriptor moves 8 KB
  contiguous; 4 x 1MB groups alternate between the two HWDGE queues (SP/ACT)
  for parallel descriptor generation; mcat trails the noise stream in 8
  chunks so stage 2 rides the tail of the mcat stream at matched rate.
"""

import sys

if "/opt/trn_rl_repo" not in sys.path:
    sys.path.insert(0, "/opt/trn_rl_repo")

from contextlib import ExitStack

import numpy as np

import concourse.bass as bass
import concourse.tile as tile
from concourse import mybir
from concourse.bass_utils import run_bass_kernel_spmd

B, IN, OUT, P, NS = 256, 1024, 512, 1000, 128
NCORES = 8
BSH = B // NCORES  # 32 samples per core
NG = 4             # noise DMA groups per core
GB = BSH // NG     # samples per DMA group (1 MB fp8 per dma_start)
NQ = 8             # feature chunks: 7*128 + 104 = 1000
LASTM = P - (NQ - 1) * 128  # 104
NMC = 8            # mcat DMA chunks (3 (q,v)-tiles each)

F32 = mybir.dt.float32
F16 = mybir.dt.float16
F8 = mybir.dt.float8e3
F16_NP = mybir.dt.np(F16)
F8_NP = mybir.dt.np(F8)

_CACHE = {}

LAST_RUN = None  # BassKernelResults of the most recent execution (for test.py)


def _chunk_m(q: int) -> int:
    return 128 if q < NQ - 1 else LASTM


def _split_sync_waits(nc: bass.Bass, max_waits: int = 1) -> int:
    """Walrus in this container accepts at most one sync-wait command per
    instruction. Tile emits instructions (notably the epilogue Drain and any
    op depending on two DMA queues) with several waits. Split the surplus
    onto single-wait NoOps inserted just before, on the same engine, which
    is semantically identical for sem-ge waits."""
    nid = 0
    for fn in nc.m.functions:
        for bb in fn.blocks:
            insts = list(bb.instructions)
            out, changed = [], False
            for inst in insts:
                si = inst.sync_info
                if si is not None and si.on_wait and len(si.on_wait) > max_waits:
                    waits = list(si.on_wait)
                    extra, keep = waits[:-max_waits], waits[-max_waits:]
                    for w in extra:
                        nid += 1
                        out.append(
                            mybir.InstNoOp(
                                name=f"waitsplit-{nid}",
                                sync_info=mybir.SyncInfo(on_wait=[w], on_update=[]),
                                bass_nofuse=True,
                                engine=inst.engine,
                            )
                        )
                    inst.sync_info = mybir.SyncInfo(
                        on_wait=keep, on_update=list(si.on_update)
                    )
                    changed = True
                out.append(inst)
            if changed:
                bb.instructions = out
    return nid


def _build_program(has_bias: bool) -> bass.Bass:
    key = ("nc", has_bias)
    if key in _CACHE:
        return _CACHE[key]

    nc = bass.Bass()

    noise_d = nc.dram_tensor("noise_sh", [NS, BSH, P], F8, kind="ExternalInput")
    g3_d = nc.dram_tensor("g3", [NS, 3], F16, kind="ExternalInput")
    mcata_d = nc.dram_tensor("mcata", [128, NQ, OUT], F16, kind="ExternalInput")
    mcatlf_d = nc.dram_tensor("mcatlf", [128, 2 * NQ, OUT], F8, kind="ExternalInput")
    c_d = nc.dram_tensor("cvec", [BSH, 1], F32, kind="ExternalInput")
    if has_bias:
        bias_d = nc.dram_tensor("biasv", [1, OUT], F32, kind="ExternalInput")
    out_d = nc.dram_tensor("out", [BSH, OUT], F32, kind="ExternalOutput")

    def bcast(ap: bass.AP, parts: int) -> bass.AP:
        # replicate a [1, N] DRAM row across `parts` partitions
        return bass.AP(tensor=ap.tensor, offset=ap.offset, ap=[[0, parts]] + ap.ap[1:])

    with ExitStack() as ctx:
        tc = ctx.enter_context(tile.TileContext(nc))
        consts = ctx.enter_context(tc.tile_pool(name="consts", bufs=1))
        npool = ctx.enter_context(tc.tile_pool(name="noise", bufs=NG))
        vpool = ctx.enter_context(tc.tile_pool(name="v", bufs=1))
        ps1 = ctx.enter_context(tc.tile_pool(name="ps1", bufs=4, space="PSUM"))
        ps2 = ctx.enter_context(tc.tile_pool(name="ps2", bufs=1, space="PSUM"))

        # ---- tiny constants first on the ACT ring ----
        g3_sb = consts.tile([NS, 3], F16, tag="g3")
        nc.scalar.dma_start(out=g3_sb[:], in_=g3_d[:])
        c_sb = consts.tile([BSH, 1], F32, tag="c")
        nc.scalar.dma_start(out=c_sb[:], in_=c_d[:])
        if has_bias:
            bias_sb = consts.tile([BSH, OUT], F32, tag="bias")
            nc.scalar.dma_start(out=bias_sb[:], in_=bcast(bias_d[:], BSH))

        # ---- bulk input stream: ONE ordered queue (SP) so noise strictly
        # precedes mcat and the DMA engines never interleave the two.
        # 9 bulk instructions <= 10 DMA semaphores: no sem-reuse stalls.
        # Host pre-transposed noise to [s, b, i] so every descriptor is 8KB.
        noise_t = []
        for g in range(NG):
            t = npool.tile([NS, GB, P], F8, name=f"noise{g}", tag="noise")
            nc.sync.dma_start(out=t[:], in_=noise_d[:][:, g * GB : (g + 1) * GB, :])
            noise_t.append(t)

        # mcat trails the noise on the same queue. The dominant A block (Tsum)
        # stays fp16; the L/F blocks ride fp8 (their error contribution is
        # share-weighted by ||gL||/||gA|| ~ 1/75). Stage 2 consumes tiles in
        # stream order (all A, all L, all F); the last chunk is one tile so
        # the final dependency lands almost with the stream end.
        # fp8 L/F tiles first (PE consumes them slower than DMA delivers),
        # fp16 A tiles last, final A chunk split off so the last dependency
        # is one tile. Stage 2 accumulates in the same L, F, A order.
        mcatlf_sb = consts.tile([128, 2 * NQ, OUT], F8, tag="mcatlf")
        for lo, hi in ((0, NQ), (NQ, 2 * NQ)):
            nc.sync.dma_start(out=mcatlf_sb[:, lo:hi, :],
                              in_=mcatlf_d[:][:, lo:hi, :])
        mcata_sb = consts.tile([128, NQ, OUT], F16, tag="mcata")
        nc.sync.dma_start(out=mcata_sb[:, 0 : NQ - 1, :],
                          in_=mcata_d[:][:, 0 : NQ - 1, :])
        nc.sync.dma_start(out=mcata_sb[:, NQ - 1 : NQ, :],
                          in_=mcata_d[:][:, NQ - 1 : NQ, :])

        # ---- PE warm-up: the tensor engine p-state ramps to full clock only
        # after several us of sustained activity. Burn cheap 512-col matmuls
        # into a scratch psum bank while the first noise groups stream so
        # stage 1/2 run at 2.4 GHz instead of 1.2. ----
        junk_sb = consts.tile([128, OUT], F16, tag="junk")
        nc.vector.memset(junk_sb[:], 0.0)
        psd = ctx.enter_context(tc.tile_pool(name="psd", bufs=1, space="PSUM"))
        psd_t = psd.tile([1, OUT], F32, tag="psdummy")

        def warm(n):
            for _ in range(n):
                nc.tensor.matmul(psd_t[:], lhsT=junk_sb[:, 0:1], rhs=junk_sb[:],
                                 start=True, stop=True)

        warm(14)

        # ---- stage 1: one 3-col matmul per (sample, chunk): stationary is
        # the fp8 noise chunk, moving is the exact fp16 g3 (mixed dtypes).
        # psum col = qparity*(BSH*3) + b*3 + v ----
        ps1_t = [ps1.tile([128, 2 * BSH * 3], F32, name=f"ps1_{i}", tag="ps1")
                 for i in range(4)]
        for g in range(NG):
            for bl in range(GB):
                b = g * GB + bl
                for q in range(NQ):
                    m = _chunk_m(q)
                    co = (q % 2) * (BSH * 3) + b * 3
                    nc.tensor.matmul(
                        ps1_t[q // 2][0:m, co : co + 3],
                        lhsT=noise_t[g][:, bl, q * 128 : q * 128 + m],
                        rhs=g3_sb[:],
                        start=True,
                        stop=True,
                    )
            warm(2)  # bridge the DMA pacing gap, keep the clock up

        # ---- psum -> V tiles (fp16): add the hi/lo pairs, reorder
        # (b,h,v) -> (v,b) ----
        v_t = [vpool.tile([128, 3 * BSH], F16, name=f"v{q}", tag=f"v{q}")
               for q in range(NQ)]
        v_lf = [vpool.tile([128, 2 * BSH], F8, name=f"vlf{q}", tag=f"vlf{q}")
                for q in range(NQ)]
        nc.vector.memset(v_t[NQ - 1][:], 0.0)  # zero-pad rows 104..127 of last chunk
        nc.vector.memset(v_lf[NQ - 1][:], 0.0)
        COPY = mybir.ActivationFunctionType.Copy
        for q in range(NQ):
            m = _chunk_m(q)
            src = ps1_t[q // 2][0:m, (q % 2) * (BSH * 3) : (q % 2 + 1) * (BSH * 3)]
            src = src.rearrange("p (b v) -> p v b", v=3)
            dst = v_t[q][0:m, :].rearrange("p (v b) -> p v b", v=3)
            nc.vector.tensor_scalar_mul(dst, src, 1.0)
            # fp8 staging of the small L/F functionals (L prescaled 1/8 to sit
            # in e3m4's normal range; the 8x is refolded into mcat's L rows)
            nc.scalar.activation(v_lf[q][0:m, 0:BSH], v_t[q][0:m, BSH : 2 * BSH],
                                 COPY, scale=0.125)
            nc.scalar.activation(v_lf[q][0:m, BSH : 2 * BSH],
                                 v_t[q][0:m, 2 * BSH : 3 * BSH], COPY)

        # ---- stage 2: out[b, j] accumulation: 8 fp16 A-tiles then 16 fp8
        # L/F tiles, in mcat stream order ----
        ps_out = ps2.tile([BSH, OUT], F32, tag="ps2")
        for t in range(2 * NQ):
            q, vv = t % NQ, t // NQ
            nc.tensor.matmul(ps_out[:],
                             lhsT=v_lf[q][:, vv * BSH : (vv + 1) * BSH],
                             rhs=mcatlf_sb[:, t, :],
                             start=(t == 0), stop=False)
        for q in range(NQ):
            nc.tensor.matmul(ps_out[:], lhsT=v_t[q][:, 0:BSH],
                             rhs=mcata_sb[:, q, :],
                             start=False, stop=(q == NQ - 1))

        # ---- scale by per-sample c_b (and bias if present), store.
        # j-halves pipeline the scale op against the out DMA. ----
        out_sb = consts.tile([BSH, OUT], F32, tag="outsb")
        if has_bias:
            tmp_sb = consts.tile([BSH, OUT], F32, tag="tmpsb")
            nc.vector.tensor_scalar_mul(tmp_sb[:], ps_out[:], c_sb[:])
            nc.vector.tensor_add(out_sb[:], tmp_sb[:], bias_sb[:])
            nc.scalar.dma_start(out=out_d[:], in_=out_sb[:])
        else:
            half = OUT // 2
            for h in range(2):
                sl = slice(h * half, (h + 1) * half)
                nc.vector.tensor_scalar_mul(out_sb[:, sl], ps_out[:, sl], c_sb[:])
                nc.scalar.dma_start(out=out_d[:][:, sl], in_=out_sb[:, sl])

    _split_sync_waits(nc)
    _CACHE[key] = nc
    return nc


def _quantize_noise_shaped(noise: np.ndarray) -> np.ndarray:
    """First-order error-feedback quantization to fp8 e3m4 along the time
    axis. noise: [B, NS, P] float32 -> [NS, B, P] e3m4 (time-major for the
    device DMA layout)."""
    q = np.empty((NS, B, P), dtype=F8_NP)
    carry = np.zeros((B, P), np.float32)
    for t in range(NS):
        v = noise[:, t, :] + carry
        qt = v.astype(F8_NP)
        q[t] = qt
        carry = v - qt.astype(np.float32)
    return q


def _host_precompute(decay_param, conv_w, conv_b, out_w, out_b):
    dp = float(np.asarray(decay_param).reshape(-1)[0])
    decay = 0.5 / (1.0 + np.exp(-dp))
    dt = 1.0 / NS
    retain = 1.0 - decay * dt

    s = np.arange(NS, dtype=np.float64)
    gA = (1.0 - retain ** (NS - s)) / (1.0 - retain)
    gL = retain ** (NS - 1 - s)

    # exact fp16 time-weights ride as the (tiny) moving operand of stage 1
    g3 = np.zeros((NS, 3), np.float64)
    g3[:, 0] = gA
    g3[:, 1] = gL
    g3[0, 2] = 1.0
    g3 = np.ascontiguousarray(g3.astype(F16_NP))

    conv_w = np.asarray(conv_w, np.float32)
    out_w = np.asarray(out_w, np.float32)
    w_sum = conv_w.sum(axis=2)
    t_sum = out_w @ w_sum              # [OUT, P]
    t0 = out_w @ conv_w[:, :, 0]
    t2 = out_w @ conv_w[:, :, 2]

    def tile_layout(r, np_dtype):
        # [K, OUT, P] -> [128, K*NQ, OUT] with tiles q-major per variant
        k = r.shape[0]
        r_pad = np.zeros((k, OUT, NQ * 128), np.float32)
        r_pad[:, :, :P] = r
        m = r_pad.reshape(k, OUT, NQ, 128).transpose(3, 0, 2, 1)  # [128,K,NQ,OUT]
        return np.ascontiguousarray(m.reshape(128, k * NQ, OUT).astype(np_dtype))

    mcata = tile_layout(t_sum[None], F16_NP)           # fp16 A block
    # L rows x8 (V_L carries the 1/8), F rows x1: both ride fp8 e3m4
    mcatlf = tile_layout(np.stack([-8.0 * t0, -t2]), F8_NP)

    bias_vec = (
        out_w @ np.asarray(conv_b, np.float32)
        + np.asarray(out_b, np.float32).reshape(OUT)
    )
    return g3, mcata, mcatlf, bias_vec


def kernel(x, noise, fb_w, fb_b, decay_param, conv_w, conv_b, out_w, out_b,
           _trace=False):
    global LAST_RUN

    x = np.asarray(x, np.float32)
    noise_q = _quantize_noise_shaped(np.asarray(noise, np.float32))  # [NS, B, P]

    g3, mcata, mcatlf, bias_vec = _host_precompute(
        decay_param, conv_w, conv_b, out_w, out_b)
    has_bias = bool(np.any(bias_vec != 0.0))
    bias_vec = np.ascontiguousarray(bias_vec.reshape(1, OUT))

    # per-sample feedback scale: sigmoid(x . fb_w + fb_b) * sqrt(dt)/NS
    fb_w = np.asarray(fb_w, np.float32).reshape(IN)
    fb_b = float(np.asarray(fb_b, np.float32).reshape(-1)[0])
    z = x @ fb_w + fb_b
    cvec = (1.0 / (1.0 + np.exp(-z, dtype=np.float64))) * (np.sqrt(1.0 / NS) / NS)
    cvec = cvec.reshape(B).astype(np.float32)

    nc = _build_program(has_bias)

    in_maps = []
    for c in range(NCORES):
        sl = slice(c * BSH, (c + 1) * BSH)
        m = {
            "noise_sh": np.ascontiguousarray(noise_q[:, sl, :]),
            "g3": g3,
            "mcata": mcata,
            "mcatlf": mcatlf,
            "cvec": np.ascontiguousarray(cvec[sl].reshape(BSH, 1)),
        }
        if has_bias:
            m["biasv"] = bias_vec
        in_maps.append(m)

    res = run_bass_kernel_spmd(nc, in_maps, core_ids=list(range(NCORES)),
                               trace=_trace)
    LAST_RUN = res
    out = np.concatenate([m["out"] for m in res.results], axis=0)
    return out.astype(np.float32)


# revision 25
# speedup vs baseline: 1.0155x; 1.0155x over previous
"""Trainium2 Bass kernel for nn_BICEPNeuralLayer.

Math: the reference module (Euler-Maruyama SDE scan -> Conv1d over time ->
time-mean -> linear projection) is LINEAR in the noise tensor, so the whole
pipeline collapses algebraically:

  paths[t] = c_b * sum_s retain^(t-s) eps_s          (c_b = feedback_b*sqrt(dt))
  mean_t(conv(paths)) folds to per-timestep weights on eps:
     out[b] = c_b/NS * (Tsum @ A[b] - T0 @ L[b] - T2 @ F[b]) + bias
  A[b,i] = sum_s gA[s] noise[b,s,i],   gA[s] = (1-retain^(NS-s))/(1-retain)
  L[b,i] = sum_s gL[s] noise[b,s,i],   gL[s] = retain^(NS-1-s)
  F[b,i] = noise[b,0,i]
  Tsum = out_w @ (W0+W1+W2), T0 = out_w @ W0, T2 = out_w @ W2  (Wk = conv_w[:,:,k])
  bias  = out_w @ conv_b + out_b

Device work per core (pure data parallel over batch, 32 samples/core):

  noise rides HBM as fp8 e3m4, quantized on the host with FIRST-ORDER ERROR
  FEEDBACK along the time axis s (q[s] = Q(n[s]+carry)). All three time
  functionals (gA, gL, delta_0) have smooth or tiny weight profiles along s,
  so the noise-shaped quantization error cancels ~70x in the sums: end-to-end
  rel err ~6e-4 (vs 1.4e-2 for plain fp8) at HALF the fp16 HBM traffic.

  stage 1: per (sample, feature-chunk): one matmul lhsT=noise[128s x <=128i]
           (fp8 stationary), rhs=g3[128s x 3] in exact fp16 (mixed-dtype
           matmul) -> psum[i, (b,v)].
  convert: DVE copies psum -> V tiles [128i, (v,b)] fp16; ACT restages the
           small L/F functionals to fp8 (L prescaled 1/8).
  stage 2: 24 accumulating matmuls lhsT=V[128i x 32b], rhs=mcat[128i x 512j]
           fp16 (sA refolded into the Tsum rows) -> psum[32b, 512j], then one
           tensor_scalar multiply by per-sample c_b (+ bias path if nonzero)
           and DMA out.

  DMA: noise host-pre-transposed to [s, b, i] so each descriptor moves 8 KB
  contiguous; 4 x 1MB groups alternate between the two HWDGE queues (SP/ACT)
  for parallel descriptor generation; mcat trails the noise stream in 8
  chunks so stage 2 rides the tail of the mcat stream at matched rate.
"""

import sys

if "/opt/trn_rl_repo" not in sys.path:
    sys.path.insert(0, "/opt/trn_rl_repo")

from contextlib import ExitStack

import numpy as np

import concourse.bass as bass
import concourse.tile as tile
from concourse import mybir
from concourse.bass_utils import run_bass_kernel_spmd

B, IN, OUT, P, NS = 256, 1024, 512, 1000, 128
NCORES = 8
BSH = B // NCORES  # 32 samples per core
NG = 8             # noise DMA groups per core
GB = BSH // NG     # samples per DMA group (0.5 MB fp8 per dma_start)
NQ = 8             # feature chunks: 7*128 + 104 = 1000
LASTM = P - (NQ - 1) * 128  # 104
NMC = 8            # mcat DMA chunks (3 (q,v)-tiles each)

F32 = mybir.dt.float32
F16 = mybir.dt.float16
F8 = mybir.dt.float8e3
F16_NP = mybir.dt.np(F16)
F8_NP = mybir.dt.np(F8)

_CACHE = {}

LAST_RUN = None  # BassKernelResults of the most recent execution (for test.py)


def _chunk_m(q: int) -> int:
    return 128 if q < NQ - 1 else LASTM


def _split_sync_waits(nc: bass.Bass, max_waits: int = 1) -> int:
    """Walrus in this container accepts at most one sync-wait command per
    instruction. Tile emits instructions (notably the epilogue Drain and any
    op depending on two DMA queues) with several waits. Split the surplus
    onto single-wait NoOps inserted just before, on the same engine, which
    is semantically identical for sem-ge waits."""
    nid = 0
    for fn in nc.m.functions:
        for bb in fn.blocks:
            insts = list(bb.instructions)
            out, changed = [], False
            for inst in insts:
                si = inst.sync_info
                if si is not None and si.on_wait and len(si.on_wait) > max_waits:
                    waits = list(si.on_wait)
                    extra, keep = waits[:-max_waits], waits[-max_waits:]
                    for w in extra:
                        nid += 1
                        out.append(
                            mybir.InstNoOp(
                                name=f"waitsplit-{nid}",
                                sync_info=mybir.SyncInfo(on_wait=[w], on_update=[]),
                                bass_nofuse=True,
                                engine=inst.engine,
                            )
                        )
                    inst.sync_info = mybir.SyncInfo(
                        on_wait=keep, on_update=list(si.on_update)
                    )
                    changed = True
                out.append(inst)
            if changed:
                bb.instructions = out
    return nid


def _build_program(has_bias: bool) -> bass.Bass:
    key = ("nc", has_bias)
    if key in _CACHE:
        return _CACHE[key]

    nc = bass.Bass()

    noise_d = nc.dram_tensor("noise_sh", [NS, BSH, P], F8, kind="ExternalInput")
    g3_d = nc.dram_tensor("g3", [NS, 3], F16, kind="ExternalInput")
    mcata_d = nc.dram_tensor("mcata", [128, NQ, OUT], F16, kind="ExternalInput")
    mcatlf_d = nc.dram_tensor("mcatlf", [128, 2 * NQ, OUT], F8, kind="ExternalInput")
    c_d = nc.dram_tensor("cvec", [BSH, 1], F32, kind="ExternalInput")
    if has_bias:
        bias_d = nc.dram_tensor("biasv", [1, OUT], F32, kind="ExternalInput")
    out_d = nc.dram_tensor("out", [BSH, OUT], F32, kind="ExternalOutput")

    def bcast(ap: bass.AP, parts: int) -> bass.AP:
        # replicate a [1, N] DRAM row across `parts` partitions
        return bass.AP(tensor=ap.tensor, offset=ap.offset, ap=[[0, parts]] + ap.ap[1:])

    with ExitStack() as ctx:
        tc = ctx.enter_context(tile.TileContext(nc))
        consts = ctx.enter_context(tc.tile_pool(name="consts", bufs=1))
        npool = ctx.enter_context(tc.tile_pool(name="noise", bufs=NG))
        vpool = ctx.enter_context(tc.tile_pool(name="v", bufs=1))
        ps1 = ctx.enter_context(tc.tile_pool(name="ps1", bufs=4, space="PSUM"))
        ps2 = ctx.enter_context(tc.tile_pool(name="ps2", bufs=1, space="PSUM"))

        # ---- tiny constants first on the ACT ring ----
        g3_sb = consts.tile([NS, 3], F16, tag="g3")
        nc.scalar.dma_start(out=g3_sb[:], in_=g3_d[:])
        c_sb = consts.tile([BSH, 1], F32, tag="c")
        nc.scalar.dma_start(out=c_sb[:], in_=c_d[:])
        if has_bias:
            bias_sb = consts.tile([BSH, OUT], F32, tag="bias")
            nc.scalar.dma_start(out=bias_sb[:], in_=bcast(bias_d[:], BSH))

        # ---- bulk input stream: ONE ordered queue (SP) so noise strictly
        # precedes mcat and the DMA engines never interleave the two.
        # 9 bulk instructions <= 10 DMA semaphores: no sem-reuse stalls.
        # Host pre-transposed noise to [s, b, i] so every descriptor is 8KB.
        noise_t = []
        for g in range(NG):
            t = npool.tile([NS, GB, P], F8, name=f"noise{g}", tag="noise")
            nc.sync.dma_start(out=t[:], in_=noise_d[:][:, g * GB : (g + 1) * GB, :])
            noise_t.append(t)

        # mcat trails the noise on the same queue. The dominant A block (Tsum)
        # stays fp16; the L/F blocks ride fp8 (their error contribution is
        # share-weighted by ||gL||/||gA|| ~ 1/75). Stage 2 consumes tiles in
        # stream order (all A, all L, all F); the last chunk is one tile so
        # the final dependency lands almost with the stream end.
        # fp16 A tiles first (stage 2 starts on them at noise-end), fp8 L/F
        # after, oversharded into small chunks: straggler DMA engines delay
        # each chunk-completion semaphore, so finer chunks release stage-2
        # tiles sooner. Stage 2 accumulates in the same A, L, F order.
        mcata_sb = consts.tile([128, NQ, OUT], F16, tag="mcata")
        for lo, hi in ((0, 4), (4, NQ)):
            nc.sync.dma_start(out=mcata_sb[:, lo:hi, :],
                              in_=mcata_d[:][:, lo:hi, :])
        mcatlf_sb = consts.tile([128, 2 * NQ, OUT], F8, tag="mcatlf")
        for lo, hi in ((0, 4), (4, 8), (8, 12), (12, 15), (15, 16)):
            nc.sync.dma_start(out=mcatlf_sb[:, lo:hi, :],
                              in_=mcatlf_d[:][:, lo:hi, :])

        # ---- PE warm-up: the tensor engine p-state ramps to full clock only
        # after several us of sustained activity. Burn cheap 512-col matmuls
        # into a scratch psum bank while the first noise groups stream so
        # stage 1/2 run at 2.4 GHz instead of 1.2. ----
        junk_sb = consts.tile([128, OUT], F16, tag="junk")
        nc.vector.memset(junk_sb[:], 0.0)
        psd = ctx.enter_context(tc.tile_pool(name="psd", bufs=1, space="PSUM"))
        psd_t = psd.tile([1, OUT], F32, tag="psdummy")

        def warm(n):
            for _ in range(n):
                nc.tensor.matmul(psd_t[:], lhsT=junk_sb[:, 0:1], rhs=junk_sb[:],
                                 start=True, stop=True)

        warm(14)

        # ---- stage 1: one 3-col matmul per (sample, chunk): stationary is
        # the fp8 noise chunk, moving is the exact fp16 g3 (mixed dtypes).
        # psum col = qparity*(BSH*3) + b*3 + v ----
        ps1_t = [ps1.tile([128, 2 * BSH * 3], F32, name=f"ps1_{i}", tag="ps1")
                 for i in range(4)]
        def s1_mm(g, bl, q):
            b = g * GB + bl
            m = _chunk_m(q)
            co = (q % 2) * (BSH * 3) + b * 3
            nc.tensor.matmul(
                ps1_t[q // 2][0:m, co : co + 3],
                lhsT=noise_t[g][:, bl, q * 128 : q * 128 + m],
                rhs=g3_sb[:],
                start=True,
                stop=True,
            )

        for g in range(NG - 1):
            for bl in range(GB):
                for q in range(NQ):
                    s1_mm(g, bl, q)
            warm(1)  # bridge the DMA pacing gap, keep the clock up
        # last group q-outer: each chunk's psum column block completes early,
        # so the V converts pipeline against the remaining matmuls
        for q in range(NQ):
            for bl in range(GB):
                s1_mm(NG - 1, bl, q)

        # ---- psum -> V tiles (fp16): add the hi/lo pairs, reorder
        # (b,h,v) -> (v,b) ----
        v_t = [vpool.tile([128, BSH], F16, name=f"v{q}", tag=f"v{q}")
               for q in range(NQ)]
        v_lf = [vpool.tile([128, 2 * BSH], F8, name=f"vlf{q}", tag=f"vlf{q}")
                for q in range(NQ)]
        nc.vector.memset(v_t[NQ - 1][:], 0.0)  # zero-pad rows 104..127 of last chunk
        nc.vector.memset(v_lf[NQ - 1][:], 0.0)
        for q in range(NQ):
            m = _chunk_m(q)
            src = ps1_t[q // 2][0:m, (q % 2) * (BSH * 3) : (q % 2 + 1) * (BSH * 3)]
            src = src.rearrange("p (b v) -> p v b", v=3)
            # A functional -> fp16 V
            nc.vector.tensor_scalar_mul(v_t[q][0:m, :], src[:, 0], 1.0)
            # L/F functionals -> fp8, prescaled 1/8 to sit in e3m4's normal
            # range (the 8x is refolded into mcat's L/F rows)
            dst = v_lf[q][0:m, :].rearrange("p (v b) -> p v b", v=2)
            nc.vector.tensor_scalar_mul(dst, src[:, 1:3], 0.125)

        # ---- stage 2: out[b, j] accumulation: 8 fp16 A-tiles then 16 fp8
        # L/F tiles, in mcat stream order ----
        ps_out = ps2.tile([BSH, OUT], F32, tag="ps2")
        for q in range(NQ):
            nc.tensor.matmul(ps_out[:], lhsT=v_t[q][:],
                             rhs=mcata_sb[:, q, :],
                             start=(q == 0), stop=False)
        for t in range(2 * NQ):
            q, vv = t % NQ, t // NQ
            nc.tensor.matmul(ps_out[:],
                             lhsT=v_lf[q][:, vv * BSH : (vv + 1) * BSH],
                             rhs=mcatlf_sb[:, t, :],
                             start=False, stop=(t == 2 * NQ - 1))

        # ---- scale by per-sample c_b (and bias if present), store.
        # j-halves pipeline the scale op against the out DMA. ----
        out_sb = consts.tile([BSH, OUT], F32, tag="outsb")
        if has_bias:
            tmp_sb = consts.tile([BSH, OUT], F32, tag="tmpsb")
            nc.vector.tensor_scalar_mul(tmp_sb[:], ps_out[:], c_sb[:])
            nc.vector.tensor_add(out_sb[:], tmp_sb[:], bias_sb[:])
            nc.scalar.dma_start(out=out_d[:], in_=out_sb[:])
        else:
            half = OUT // 2
            for h in range(2):
                sl = slice(h * half, (h + 1) * half)
                nc.vector.tensor_scalar_mul(out_sb[:, sl], ps_out[:, sl], c_sb[:])
                nc.scalar.dma_start(out=out_d[:][:, sl], in_=out_sb[:, sl])

    _split_sync_waits(nc)
    _CACHE[key] = nc
    return nc


def _quantize_noise_shaped(noise: np.ndarray) -> np.ndarray:
    """First-order error-feedback quantization to fp8 e3m4 along the time
    axis. noise: [B, NS, P] float32 -> [NS, B, P] e3m4 (time-major for the
    device DMA layout)."""
    q = np.empty((NS, B, P), dtype=F8_NP)
    carry = np.zeros((B, P), np.float32)
    for t in range(NS):
        v = noise[:, t, :] + carry
        qt = v.astype(F8_NP)
        q[t] = qt
        carry = v - qt.astype(np.float32)
    return q


def _host_precompute(decay_param, conv_w, conv_b, out_w, out_b):
    dp = float(np.asarray(decay_param).reshape(-1)[0])
    decay = 0.5 / (1.0 + np.exp(-dp))
    dt = 1.0 / NS
    retain = 1.0 - decay * dt

    s = np.arange(NS, dtype=np.float64)
    gA = (1.0 - retain ** (NS - s)) / (1.0 - retain)
    gL = retain ** (NS - 1 - s)

    # exact fp16 time-weights ride as the (tiny) moving operand of stage 1
    g3 = np.zeros((NS, 3), np.float64)
    g3[:, 0] = gA
    g3[:, 1] = gL
    g3[0, 2] = 1.0
    g3 = np.ascontiguousarray(g3.astype(F16_NP))

    conv_w = np.asarray(conv_w, np.float32)
    out_w = np.asarray(out_w, np.float32)
    w_sum = conv_w.sum(axis=2)
    t_sum = out_w @ w_sum              # [OUT, P]
    t0 = out_w @ conv_w[:, :, 0]
    t2 = out_w @ conv_w[:, :, 2]

    def tile_layout(r, np_dtype):
        # [K, OUT, P] -> [128, K*NQ, OUT] with tiles q-major per variant
        k = r.shape[0]
        r_pad = np.zeros((k, OUT, NQ * 128), np.float32)
        r_pad[:, :, :P] = r
        m = r_pad.reshape(k, OUT, NQ, 128).transpose(3, 0, 2, 1)  # [128,K,NQ,OUT]
        return np.ascontiguousarray(m.reshape(128, k * NQ, OUT).astype(np_dtype))

    mcata = tile_layout(t_sum[None], F16_NP)           # fp16 A block
    # L rows x8 (V_L carries the 1/8), F rows x1: both ride fp8 e3m4
    mcatlf = tile_layout(np.stack([-8.0 * t0, -8.0 * t2]), F8_NP)

    bias_vec = (
        out_w @ np.asarray(conv_b, np.float32)
        + np.asarray(out_b, np.float32).reshape(OUT)
    )
    return g3, mcata, mcatlf, bias_vec


def kernel(x, noise, fb_w, fb_b, decay_param, conv_w, conv_b, out_w, out_b,
           _trace=False):
    global LAST_RUN

    x = np.asarray(x, np.float32)
    noise_q = _quantize_noise_shaped(np.asarray(noise, np.float32))  # [NS, B, P]

    g3, mcata, mcatlf, bias_vec = _host_precompute(
        decay_param, conv_w, conv_b, out_w, out_b)
    has_bias = bool(np.any(bias_vec != 0.0))
    bias_vec = np.ascontiguousarray(bias_vec.reshape(1, OUT))

    # per-sample feedback scale: sigmoid(x . fb_w + fb_b) * sqrt(dt)/NS
    fb_w = np.asarray(fb_w, np.float32).reshape(IN)
    fb_b = float(np.asarray(fb_b, np.float32).reshape(-1)[0])
    z = x @ fb_w + fb_b
    cvec = (1.0 / (1.0 + np.exp(-z, dtype=np.float64))) * (np.sqrt(1.0 / NS) / NS)
    cvec = cvec.reshape(B).astype(np.float32)

    nc = _build_program(has_bias)

    in_maps = []
    for c in range(NCORES):
        sl = slice(c * BSH, (c + 1) * BSH)
        m = {
            "noise_sh": np.ascontiguousarray(noise_q[:, sl, :]),
            "g3": g3,
            "mcata": mcata,
            "mcatlf": mcatlf,
            "cvec": np.ascontiguousarray(cvec[sl].reshape(BSH, 1)),
        }
        if has_bias:
            m["biasv"] = bias_vec
        in_maps.append(m)

    res = run_bass_kernel_spmd(nc, in_maps, core_ids=list(range(NCORES)),
                               trace=_trace)
    LAST_RUN = res
    out = np.concatenate([m["out"] for m in res.results], axis=0)
    return out.astype(np.float32)


# revision 29
# speedup vs baseline: 1.0674x; 1.0511x over previous
"""Trainium2 Bass kernel for nn_BICEPNeuralLayer.

Math: the reference module (Euler-Maruyama SDE scan -> Conv1d over time ->
time-mean -> linear projection) is LINEAR in the noise tensor, so the whole
pipeline collapses algebraically:

  paths[t] = c_b * sum_s retain^(t-s) eps_s          (c_b = feedback_b*sqrt(dt))
  mean_t(conv(paths)) folds to per-timestep weights on eps:
     out[b] = c_b/NS * (Tsum @ A[b] - T0 @ L[b] - T2 @ F[b]) + bias
  A[b,i] = sum_s gA[s] noise[b,s,i],   gA[s] = (1-retain^(NS-s))/(1-retain)
  L[b,i] = sum_s gL[s] noise[b,s,i],   gL[s] = retain^(NS-1-s)
  F[b,i] = noise[b,0,i]
  Tsum = out_w @ (W0+W1+W2), T0 = out_w @ W0, T2 = out_w @ W2  (Wk = conv_w[:,:,k])
  bias  = out_w @ conv_b + out_b

Device work per core (pure data parallel over batch, 32 samples/core):

  noise rides HBM as fp8 e3m4, quantized on the host with FIRST-ORDER ERROR
  FEEDBACK along the time axis s (q[s] = Q(n[s]+carry)). All three time
  functionals (gA, gL, delta_0) have smooth or tiny weight profiles along s,
  so the noise-shaped quantization error cancels ~70x in the sums: end-to-end
  rel err ~6e-4 (vs 1.4e-2 for plain fp8) at HALF the fp16 HBM traffic.

  stage 1: per (sample, feature-chunk): one matmul lhsT=noise[128s x <=128i]
           (fp8 stationary), rhs=g3[128s x 3] in exact fp16 (mixed-dtype
           matmul) -> psum[i, (b,v)].
  convert: DVE copies psum -> V tiles [128i, (v,b)] fp16; ACT restages the
           small L/F functionals to fp8 (L prescaled 1/8).
  stage 2: 24 accumulating matmuls lhsT=V[128i x 32b], rhs=mcat[128i x 512j]
           fp16 (sA refolded into the Tsum rows) -> psum[32b, 512j], then one
           tensor_scalar multiply by per-sample c_b (+ bias path if nonzero)
           and DMA out.

  DMA: noise host-pre-transposed to [s, b, i] so each descriptor moves 8 KB
  contiguous; 4 x 1MB groups alternate between the two HWDGE queues (SP/ACT)
  for parallel descriptor generation; mcat trails the noise stream in 8
  chunks so stage 2 rides the tail of the mcat stream at matched rate.
"""

import sys

if "/opt/trn_rl_repo" not in sys.path:
    sys.path.insert(0, "/opt/trn_rl_repo")

from contextlib import ExitStack

import numpy as np

import concourse.bass as bass
import concourse.tile as tile
from concourse import mybir
from concourse.bass_utils import run_bass_kernel_spmd

B, IN, OUT, P, NS = 256, 1024, 512, 1000, 128
NCORES = 8
BSH = B // NCORES  # 32 samples per core
NG = 8             # noise DMA groups per core
GB = BSH // NG     # samples per DMA group (0.5 MB fp8 per dma_start)
NQ = 8             # feature chunks: 7*128 + 104 = 1000
LASTM = P - (NQ - 1) * 128  # 104
NMC = 8            # mcat DMA chunks (3 (q,v)-tiles each)

F32 = mybir.dt.float32
F16 = mybir.dt.float16
F8 = mybir.dt.float8e3
F16_NP = mybir.dt.np(F16)
F8_NP = mybir.dt.np(F8)

_CACHE = {}

LAST_RUN = None  # BassKernelResults of the most recent execution (for test.py)


def _chunk_m(q: int) -> int:
    return 128 if q < NQ - 1 else LASTM


def _split_sync_waits(nc: bass.Bass, max_waits: int = 1) -> int:
    """Walrus in this container accepts at most one sync-wait command per
    instruction. Tile emits instructions (notably the epilogue Drain and any
    op depending on two DMA queues) with several waits. Split the surplus
    onto single-wait NoOps inserted just before, on the same engine, which
    is semantically identical for sem-ge waits."""
    nid = 0
    for fn in nc.m.functions:
        for bb in fn.blocks:
            insts = list(bb.instructions)
            out, changed = [], False
            for inst in insts:
                si = inst.sync_info
                if si is not None and si.on_wait and len(si.on_wait) > max_waits:
                    waits = list(si.on_wait)
                    extra, keep = waits[:-max_waits], waits[-max_waits:]
                    for w in extra:
                        nid += 1
                        out.append(
                            mybir.InstNoOp(
                                name=f"waitsplit-{nid}",
                                sync_info=mybir.SyncInfo(on_wait=[w], on_update=[]),
                                bass_nofuse=True,
                                engine=inst.engine,
                            )
                        )
                    inst.sync_info = mybir.SyncInfo(
                        on_wait=keep, on_update=list(si.on_update)
                    )
                    changed = True
                out.append(inst)
            if changed:
                bb.instructions = out
    return nid


def _build_program(has_bias: bool) -> bass.Bass:
    key = ("nc", has_bias)
    if key in _CACHE:
        return _CACHE[key]

    nc = bass.Bass()

    noise_d = nc.dram_tensor("noise_sh", [NS, BSH, P], F8, kind="ExternalInput")
    g3_d = nc.dram_tensor("g3", [NS, 3], F16, kind="ExternalInput")
    NJC = OUT // 128   # stage-2 output row chunks (j on psum partitions)
    mcata_d = nc.dram_tensor("mcata", [128, NQ, NJC, 128], F16, kind="ExternalInput")
    mcatlf_d = nc.dram_tensor("mcatlf", [128, 2 * NQ, NJC, 128], F8,
                              kind="ExternalInput")
    c_d = nc.dram_tensor("cvec", [1, BSH], F32, kind="ExternalInput")
    if has_bias:
        bias_d = nc.dram_tensor("biasv", [OUT, 1], F32, kind="ExternalInput")
    out_d = nc.dram_tensor("out", [OUT, BSH], F32, kind="ExternalOutput")

    def bcast(ap: bass.AP, parts: int) -> bass.AP:
        # replicate a [1, N] DRAM row across `parts` partitions
        return bass.AP(tensor=ap.tensor, offset=ap.offset, ap=[[0, parts]] + ap.ap[1:])

    with ExitStack() as ctx:
        tc = ctx.enter_context(tile.TileContext(nc))
        consts = ctx.enter_context(tc.tile_pool(name="consts", bufs=1))
        npool = ctx.enter_context(tc.tile_pool(name="noise", bufs=NG))
        vpool = ctx.enter_context(tc.tile_pool(name="v", bufs=1))
        ps1 = ctx.enter_context(tc.tile_pool(name="ps1", bufs=2, space="PSUM"))
        ps2 = ctx.enter_context(tc.tile_pool(name="ps2", bufs=1, space="PSUM"))

        # ---- tiny constants first on the ACT ring ----
        g3_sb = consts.tile([NS, 3], F16, tag="g3")
        nc.scalar.dma_start(out=g3_sb[:], in_=g3_d[:])
        c_sb = consts.tile([128, BSH], F32, tag="c")
        nc.scalar.dma_start(out=c_sb[:], in_=bcast(c_d[:], 128))
        if has_bias:
            # bias[j] per psum partition: [128, NJC] with column jc
            bias_sb = consts.tile([128, NJC], F32, tag="bias")
            nc.scalar.dma_start(out=bias_sb[:],
                                in_=bias_d[:].rearrange("(jc j) o -> j (jc o)", jc=NJC))

        # ---- bulk input stream: ONE ordered queue (SP) so noise strictly
        # precedes mcat and the DMA engines never interleave the two.
        # 9 bulk instructions <= 10 DMA semaphores: no sem-reuse stalls.
        # Host pre-transposed noise to [s, b, i] so every descriptor is 8KB.
        noise_t = []
        for g in range(NG):
            t = npool.tile([NS, GB, P], F8, name=f"noise{g}", tag="noise")
            nc.sync.dma_start(out=t[:], in_=noise_d[:][:, g * GB : (g + 1) * GB, :])
            noise_t.append(t)

        # mcat trails the noise on the same queue. The dominant A block (Tsum)
        # stays fp16; the L/F blocks ride fp8 (their error contribution is
        # share-weighted by ||gL||/||gA|| ~ 1/75). Stage 2 consumes tiles in
        # stream order (all A, all L, all F); the last chunk is one tile so
        # the final dependency lands almost with the stream end.
        # fp16 A tiles first (stage 2 starts on them at noise-end), fp8 L/F
        # after, oversharded into small chunks: straggler DMA engines delay
        # each chunk-completion semaphore, so finer chunks release stage-2
        # tiles sooner. Stage 2 accumulates in the same A, L, F order.
        mcata_sb = consts.tile([128, NQ, NJC, 128], F16, tag="mcata")
        for lo, hi in ((0, 4), (4, NQ)):
            nc.sync.dma_start(out=mcata_sb[:, lo:hi, :, :],
                              in_=mcata_d[:][:, lo:hi, :, :])
        mcatlf_sb = consts.tile([128, 2 * NQ, NJC, 128], F8, tag="mcatlf")
        for lo, hi in ((0, 4), (4, 8), (8, 12), (12, 15), (15, 16)):
            nc.sync.dma_start(out=mcatlf_sb[:, lo:hi, :, :],
                              in_=mcatlf_d[:][:, lo:hi, :, :])

        # ---- PE warm-up: the tensor engine p-state ramps to full clock only
        # after several us of sustained activity. Burn cheap 512-col matmuls
        # into a scratch psum bank while the first noise groups stream so
        # stage 1/2 run at 2.4 GHz instead of 1.2. ----
        junk_sb = consts.tile([128, OUT], F16, tag="junk")
        nc.vector.memset(junk_sb[:], 0.0)
        psd = ctx.enter_context(tc.tile_pool(name="psd", bufs=1, space="PSUM"))
        psd_t = psd.tile([1, OUT], F32, tag="psdummy")

        def warm(n):
            for _ in range(n):
                nc.tensor.matmul(psd_t[:], lhsT=junk_sb[:, 0:1], rhs=junk_sb[:],
                                 start=True, stop=True)

        warm(14)

        # ---- stage 1: one 3-col matmul per (sample, chunk): stationary is
        # the fp8 noise chunk, moving is the exact fp16 g3 (mixed dtypes).
        # psum col = qparity*(BSH*3) + b*3 + v ----
        ps1_t = [ps1.tile([128, 4 * BSH * 3], F32, name=f"ps1_{i}", tag="ps1")
                 for i in range(2)]
        def s1_mm(g, bl, q):
            b = g * GB + bl
            m = _chunk_m(q)
            co = (q % 4) * (BSH * 3) + b * 3
            nc.tensor.matmul(
                ps1_t[q // 4][0:m, co : co + 3],
                lhsT=noise_t[g][:, bl, q * 128 : q * 128 + m],
                rhs=g3_sb[:],
                start=True,
                stop=True,
            )

        for g in range(NG - 1):
            for bl in range(GB):
                for q in range(NQ):
                    s1_mm(g, bl, q)
            warm(2)  # bridge the DMA pacing gap, keep the clock up
        # last group q-outer: each chunk's psum column block completes early,
        # so the V converts pipeline against the remaining matmuls
        for q in range(NQ):
            for bl in range(GB):
                s1_mm(NG - 1, bl, q)

        # ---- psum -> V tiles (fp16): add the hi/lo pairs, reorder
        # (b,h,v) -> (v,b) ----
        v_t = [vpool.tile([128, BSH], F16, name=f"v{q}", tag=f"v{q}")
               for q in range(NQ)]
        v_lf = [vpool.tile([128, 2 * BSH], F8, name=f"vlf{q}", tag=f"vlf{q}")
                for q in range(NQ)]
        nc.vector.memset(v_t[NQ - 1][:], 0.0)  # zero-pad rows 104..127 of last chunk
        nc.vector.memset(v_lf[NQ - 1][:], 0.0)
        for q in range(NQ):
            m = _chunk_m(q)
            src = ps1_t[q // 4][0:m, (q % 4) * (BSH * 3) : (q % 4 + 1) * (BSH * 3)]
            src = src.rearrange("p (b v) -> p v b", v=3)
            # A functional -> fp16 V
            nc.vector.tensor_scalar_mul(v_t[q][0:m, :], src[:, 0], 1.0)
            # L/F functionals -> fp8, prescaled 1/8 to sit in e3m4's normal
            # range (the 8x is refolded into mcat's L/F rows)
            dst = v_lf[q][0:m, :].rearrange("p (v b) -> p v b", v=2)
            nc.vector.tensor_scalar_mul(dst, src[:, 1:3], 0.125)

        # ---- stage 2: out[b, j] accumulation: 8 fp16 A-tiles then 16 fp8
        # L/F tiles, in mcat stream order ----
        psj = [ps2.tile([128, BSH], F32, name=f"psj{jc}", tag=f"psj{jc}")
               for jc in range(NJC)]
        for q in range(NQ):
            for jc in range(NJC):
                nc.tensor.matmul(psj[jc][:],
                                 lhsT=mcata_sb[:, q, jc, :],
                                 rhs=v_t[q][:],
                                 start=(q == 0), stop=False)
        for t in range(2 * NQ):
            q, vv = t % NQ, t // NQ
            for jc in range(NJC):
                nc.tensor.matmul(psj[jc][:],
                                 lhsT=mcatlf_sb[:, t, jc, :],
                                 rhs=v_lf[q][:, vv * BSH : (vv + 1) * BSH],
                                 start=False,
                                 stop=(t == 2 * NQ - 1))

        # ---- scale by per-sample c_b (columns), store transposed [OUT, BSH];
        # per-jc ops pipeline the scale against the out DMA ----
        out_sb = consts.tile([128, NJC, BSH], F32, tag="outsb")
        out_view = out_d[:].rearrange("(jc j) b -> j jc b", jc=NJC)
        for jc in range(NJC):
            nc.vector.tensor_mul(out_sb[:, jc, :], psj[jc][:], c_sb[:])
            if has_bias:
                nc.vector.tensor_scalar_add(out_sb[:, jc, :], out_sb[:, jc, :],
                                            bias_sb[:, jc : jc + 1])
            nc.scalar.dma_start(out=out_view[:, jc, :], in_=out_sb[:, jc, :])

    _split_sync_waits(nc)
    _CACHE[key] = nc
    return nc


def _quantize_noise_shaped(noise: np.ndarray) -> np.ndarray:
    """First-order error-feedback quantization to fp8 e3m4 along the time
    axis. noise: [B, NS, P] float32 -> [NS, B, P] e3m4 (time-major for the
    device DMA layout)."""
    q = np.empty((NS, B, P), dtype=F8_NP)
    carry = np.zeros((B, P), np.float32)
    for t in range(NS):
        v = noise[:, t, :] + carry
        qt = v.astype(F8_NP)
        q[t] = qt
        carry = v - qt.astype(np.float32)
    return q


def _host_precompute(decay_param, conv_w, conv_b, out_w, out_b):
    dp = float(np.asarray(decay_param).reshape(-1)[0])
    decay = 0.5 / (1.0 + np.exp(-dp))
    dt = 1.0 / NS
    retain = 1.0 - decay * dt

    s = np.arange(NS, dtype=np.float64)
    gA = (1.0 - retain ** (NS - s)) / (1.0 - retain)
    gL = retain ** (NS - 1 - s)

    # exact fp16 time-weights ride as the (tiny) moving operand of stage 1
    g3 = np.zeros((NS, 3), np.float64)
    g3[:, 0] = gA
    g3[:, 1] = gL
    g3[0, 2] = 1.0
    g3 = np.ascontiguousarray(g3.astype(F16_NP))

    conv_w = np.asarray(conv_w, np.float32)
    out_w = np.asarray(out_w, np.float32)
    w_sum = conv_w.sum(axis=2)
    t_sum = out_w @ w_sum              # [OUT, P]
    t0 = out_w @ conv_w[:, :, 0]
    t2 = out_w @ conv_w[:, :, 2]

    def tile_layout(r, np_dtype):
        # [K, OUT, P] -> [128, K*NQ, NJC, 128]: stationary tiles [128i, 128j]
        k = r.shape[0]
        njc = OUT // 128
        r_pad = np.zeros((k, OUT, NQ * 128), np.float32)
        r_pad[:, :, :P] = r
        m = r_pad.reshape(k, njc, 128, NQ, 128).transpose(4, 0, 3, 1, 2)
        return np.ascontiguousarray(
            m.reshape(128, k * NQ, njc, 128).astype(np_dtype))

    mcata = tile_layout(t_sum[None], F16_NP)           # fp16 A block
    # L rows x8 (V_L carries the 1/8), F rows x1: both ride fp8 e3m4
    mcatlf = tile_layout(np.stack([-8.0 * t0, -8.0 * t2]), F8_NP)

    bias_vec = (
        out_w @ np.asarray(conv_b, np.float32)
        + np.asarray(out_b, np.float32).reshape(OUT)
    )
    return g3, mcata, mcatlf, bias_vec


def kernel(x, noise, fb_w, fb_b, decay_param, conv_w, conv_b, out_w, out_b,
           _trace=False):
    global LAST_RUN

    x = np.asarray(x, np.float32)
    noise_q = _quantize_noise_shaped(np.asarray(noise, np.float32))  # [NS, B, P]

    g3, mcata, mcatlf, bias_vec = _host_precompute(
        decay_param, conv_w, conv_b, out_w, out_b)
    has_bias = bool(np.any(bias_vec != 0.0))

    # per-sample feedback scale: sigmoid(x . fb_w + fb_b) * sqrt(dt)/NS
    fb_w = np.asarray(fb_w, np.float32).reshape(IN)
    fb_b = float(np.asarray(fb_b, np.float32).reshape(-1)[0])
    z = x @ fb_w + fb_b
    cvec = (1.0 / (1.0 + np.exp(-z, dtype=np.float64))) * (np.sqrt(1.0 / NS) / NS)
    cvec = cvec.reshape(B).astype(np.float32)

    nc = _build_program(has_bias)

    in_maps = []
    for c in range(NCORES):
        sl = slice(c * BSH, (c + 1) * BSH)
        m = {
            "noise_sh": np.ascontiguousarray(noise_q[:, sl, :]),
            "g3": g3,
            "mcata": mcata,
            "mcatlf": mcatlf,
            "cvec": np.ascontiguousarray(cvec[sl].reshape(1, BSH)),
        }
        if has_bias:
            m["biasv"] = np.ascontiguousarray(bias_vec.reshape(OUT, 1))
        in_maps.append(m)

    res = run_bass_kernel_spmd(nc, in_maps, core_ids=list(range(NCORES)),
                               trace=_trace)
    LAST_RUN = res
    out = np.concatenate([m["out"].T for m in res.results], axis=0)
    return out.astype(np.float32)


# revision 30
# speedup vs baseline: 1.0785x; 1.0104x over previous
"""Trainium2 Bass kernel for nn_BICEPNeuralLayer.

Math: the reference module (Euler-Maruyama SDE scan -> Conv1d over time ->
time-mean -> linear projection) is LINEAR in the noise tensor, so the whole
pipeline collapses algebraically:

  paths[t] = c_b * sum_s retain^(t-s) eps_s          (c_b = feedback_b*sqrt(dt))
  mean_t(conv(paths)) folds to per-timestep weights on eps:
     out[b] = c_b/NS * (Tsum @ A[b] - T0 @ L[b] - T2 @ F[b]) + bias
  A[b,i] = sum_s gA[s] noise[b,s,i],   gA[s] = (1-retain^(NS-s))/(1-retain)
  L[b,i] = sum_s gL[s] noise[b,s,i],   gL[s] = retain^(NS-1-s)
  F[b,i] = noise[b,0,i]
  Tsum = out_w @ (W0+W1+W2), T0 = out_w @ W0, T2 = out_w @ W2  (Wk = conv_w[:,:,k])
  bias  = out_w @ conv_b + out_b

Device work per core (pure data parallel over batch, 32 samples/core):

  noise rides HBM as fp8 e3m4, quantized on the host with FIRST-ORDER ERROR
  FEEDBACK along the time axis s (q[s] = Q(n[s]+carry)). All three time
  functionals (gA, gL, delta_0) have smooth or tiny weight profiles along s,
  so the noise-shaped quantization error cancels ~70x in the sums: end-to-end
  rel err ~6e-4 (vs 1.4e-2 for plain fp8) at HALF the fp16 HBM traffic.

  stage 1: per (sample, feature-chunk): one matmul lhsT=noise[128s x <=128i]
           (fp8 stationary), rhs=g3[128s x 3] in exact fp16 (mixed-dtype
           matmul) -> psum[i, (b,v)].
  convert: DVE copies psum -> V tiles [128i, (v,b)] fp16; ACT restages the
           small L/F functionals to fp8 (L prescaled 1/8).
  stage 2: 24 accumulating matmuls lhsT=V[128i x 32b], rhs=mcat[128i x 512j]
           fp16 (sA refolded into the Tsum rows) -> psum[32b, 512j], then one
           tensor_scalar multiply by per-sample c_b (+ bias path if nonzero)
           and DMA out.

  DMA: noise host-pre-transposed to [s, b, i] so each descriptor moves 8 KB
  contiguous; 4 x 1MB groups alternate between the two HWDGE queues (SP/ACT)
  for parallel descriptor generation; mcat trails the noise stream in 8
  chunks so stage 2 rides the tail of the mcat stream at matched rate.
"""

import sys

if "/opt/trn_rl_repo" not in sys.path:
    sys.path.insert(0, "/opt/trn_rl_repo")

from contextlib import ExitStack

import numpy as np

import concourse.bass as bass
import concourse.tile as tile
from concourse import mybir
from concourse.bass_utils import run_bass_kernel_spmd

B, IN, OUT, P, NS = 256, 1024, 512, 1000, 128
NCORES = 8
BSH = B // NCORES  # 32 samples per core
NG = 8             # noise DMA groups per core
GB = BSH // NG     # samples per DMA group (0.5 MB fp8 per dma_start)
NQ = 8             # feature chunks: 7*128 + 104 = 1000
LASTM = P - (NQ - 1) * 128  # 104
NMC = 8            # mcat DMA chunks (3 (q,v)-tiles each)

F32 = mybir.dt.float32
F16 = mybir.dt.float16
F8 = mybir.dt.float8e3
F16_NP = mybir.dt.np(F16)
F8_NP = mybir.dt.np(F8)

_CACHE = {}

LAST_RUN = None  # BassKernelResults of the most recent execution (for test.py)


def _chunk_m(q: int) -> int:
    return 128 if q < NQ - 1 else LASTM


def _split_sync_waits(nc: bass.Bass, max_waits: int = 1) -> int:
    """Walrus in this container accepts at most one sync-wait command per
    instruction. Tile emits instructions (notably the epilogue Drain and any
    op depending on two DMA queues) with several waits. Split the surplus
    onto single-wait NoOps inserted just before, on the same engine, which
    is semantically identical for sem-ge waits."""
    nid = 0
    for fn in nc.m.functions:
        for bb in fn.blocks:
            insts = list(bb.instructions)
            out, changed = [], False
            for inst in insts:
                si = inst.sync_info
                if si is not None and si.on_wait and len(si.on_wait) > max_waits:
                    waits = list(si.on_wait)
                    extra, keep = waits[:-max_waits], waits[-max_waits:]
                    for w in extra:
                        nid += 1
                        out.append(
                            mybir.InstNoOp(
                                name=f"waitsplit-{nid}",
                                sync_info=mybir.SyncInfo(on_wait=[w], on_update=[]),
                                bass_nofuse=True,
                                engine=inst.engine,
                            )
                        )
                    inst.sync_info = mybir.SyncInfo(
                        on_wait=keep, on_update=list(si.on_update)
                    )
                    changed = True
                out.append(inst)
            if changed:
                bb.instructions = out
    return nid


def _build_program(has_bias: bool) -> bass.Bass:
    key = ("nc", has_bias)
    if key in _CACHE:
        return _CACHE[key]

    nc = bass.Bass()

    noise_d = nc.dram_tensor("noise_sh", [NS, BSH, P], F8, kind="ExternalInput")
    g3_d = nc.dram_tensor("g3", [NS, 3], F16, kind="ExternalInput")
    NJC = OUT // 128   # stage-2 output row chunks (j on psum partitions)
    mcata_d = nc.dram_tensor("mcata", [128, NQ, NJC, 128], F16, kind="ExternalInput")
    mcatlf_d = nc.dram_tensor("mcatlf", [128, 2 * NQ, NJC, 128], F8,
                              kind="ExternalInput")
    c_d = nc.dram_tensor("cvec", [1, BSH], F32, kind="ExternalInput")
    if has_bias:
        bias_d = nc.dram_tensor("biasv", [OUT, 1], F32, kind="ExternalInput")
    out_d = nc.dram_tensor("out", [OUT, BSH], F32, kind="ExternalOutput")

    def bcast(ap: bass.AP, parts: int) -> bass.AP:
        # replicate a [1, N] DRAM row across `parts` partitions
        return bass.AP(tensor=ap.tensor, offset=ap.offset, ap=[[0, parts]] + ap.ap[1:])

    with ExitStack() as ctx:
        tc = ctx.enter_context(tile.TileContext(nc))
        consts = ctx.enter_context(tc.tile_pool(name="consts", bufs=1))
        npool = ctx.enter_context(tc.tile_pool(name="noise", bufs=NG))
        vpool = ctx.enter_context(tc.tile_pool(name="v", bufs=1))
        ps1 = ctx.enter_context(tc.tile_pool(name="ps1", bufs=2, space="PSUM"))
        ps2 = ctx.enter_context(tc.tile_pool(name="ps2", bufs=1, space="PSUM"))

        # ---- tiny constants first on the ACT ring ----
        g3_sb = consts.tile([NS, 3], F16, tag="g3")
        nc.scalar.dma_start(out=g3_sb[:], in_=g3_d[:])
        c_sb = consts.tile([128, BSH], F32, tag="c")
        nc.scalar.dma_start(out=c_sb[:], in_=bcast(c_d[:], 128))
        if has_bias:
            # bias[j] per psum partition: [128, NJC] with column jc
            bias_sb = consts.tile([128, NJC], F32, tag="bias")
            nc.scalar.dma_start(out=bias_sb[:],
                                in_=bias_d[:].rearrange("(jc j) o -> j (jc o)", jc=NJC))

        # ---- bulk input stream: ONE ordered queue (SP) so noise strictly
        # precedes mcat and the DMA engines never interleave the two.
        # 9 bulk instructions <= 10 DMA semaphores: no sem-reuse stalls.
        # Host pre-transposed noise to [s, b, i] so every descriptor is 8KB.
        noise_t = []
        for g in range(NG):
            t = npool.tile([NS, GB, P], F8, name=f"noise{g}", tag="noise")
            nc.sync.dma_start(out=t[:], in_=noise_d[:][:, g * GB : (g + 1) * GB, :])
            noise_t.append(t)

        # mcat trails the noise on the same queue. The dominant A block (Tsum)
        # stays fp16; the L/F blocks ride fp8 (their error contribution is
        # share-weighted by ||gL||/||gA|| ~ 1/75). Stage 2 consumes tiles in
        # stream order (all A, all L, all F); the last chunk is one tile so
        # the final dependency lands almost with the stream end.
        # fp16 A tiles first (stage 2 starts on them at noise-end), fp8 L/F
        # after, oversharded into small chunks: straggler DMA engines delay
        # each chunk-completion semaphore, so finer chunks release stage-2
        # tiles sooner. Stage 2 accumulates in the same A, L, F order.
        mcata_sb = consts.tile([128, NQ, NJC, 128], F16, tag="mcata")
        for lo, hi in ((0, 4), (4, NQ)):
            nc.sync.dma_start(out=mcata_sb[:, lo:hi, :, :],
                              in_=mcata_d[:][:, lo:hi, :, :])
        mcatlf_sb = consts.tile([128, 2 * NQ, NJC, 128], F8, tag="mcatlf")
        for lo, hi in ((0, 4), (4, 8), (8, 12), (12, 15), (15, 16)):
            nc.sync.dma_start(out=mcatlf_sb[:, lo:hi, :, :],
                              in_=mcatlf_d[:][:, lo:hi, :, :])


        # ---- stage 1: one 3-col matmul per (sample, chunk): stationary is
        # the fp8 noise chunk, moving is the exact fp16 g3 (mixed dtypes).
        # psum col = qparity*(BSH*3) + b*3 + v ----
        ps1_t = [ps1.tile([128, 4 * BSH * 3], F32, name=f"ps1_{i}", tag="ps1")
                 for i in range(2)]
        def s1_mm(g, bl, q):
            b = g * GB + bl
            m = _chunk_m(q)
            co = (q % 4) * (BSH * 3) + b * 3
            nc.tensor.matmul(
                ps1_t[q // 4][0:m, co : co + 3],
                lhsT=noise_t[g][:, bl, q * 128 : q * 128 + m],
                rhs=g3_sb[:],
                start=True,
                stop=True,
            )

        for g in range(NG - 1):
            for bl in range(GB):
                for q in range(NQ):
                    s1_mm(g, bl, q)
        # last group q-outer: each chunk's psum column block completes early,
        # so the V converts pipeline against the remaining matmuls
        for q in range(NQ):
            for bl in range(GB):
                s1_mm(NG - 1, bl, q)

        # ---- psum -> V tiles (fp16): add the hi/lo pairs, reorder
        # (b,h,v) -> (v,b) ----
        v_t = [vpool.tile([128, BSH], F16, name=f"v{q}", tag=f"v{q}")
               for q in range(NQ)]
        v_lf = [vpool.tile([128, 2 * BSH], F8, name=f"vlf{q}", tag=f"vlf{q}")
                for q in range(NQ)]
        nc.vector.memset(v_t[NQ - 1][:], 0.0)  # zero-pad rows 104..127 of last chunk
        nc.vector.memset(v_lf[NQ - 1][:], 0.0)
        for q in range(NQ):
            m = _chunk_m(q)
            src = ps1_t[q // 4][0:m, (q % 4) * (BSH * 3) : (q % 4 + 1) * (BSH * 3)]
            src = src.rearrange("p (b v) -> p v b", v=3)
            # A functional -> fp16 V
            nc.vector.tensor_scalar_mul(v_t[q][0:m, :], src[:, 0], 1.0)
            # L/F functionals -> fp8, prescaled 1/8 to sit in e3m4's normal
            # range (the 8x is refolded into mcat's L/F rows)
            dst = v_lf[q][0:m, :].rearrange("p (v b) -> p v b", v=2)
            nc.vector.tensor_scalar_mul(dst, src[:, 1:3], 0.125)

        # ---- stage 2: out[b, j] accumulation: 8 fp16 A-tiles then 16 fp8
        # L/F tiles, in mcat stream order ----
        psj = [ps2.tile([128, BSH], F32, name=f"psj{jc}", tag=f"psj{jc}")
               for jc in range(NJC)]
        for q in range(NQ):
            for jc in range(NJC):
                nc.tensor.matmul(psj[jc][:],
                                 lhsT=mcata_sb[:, q, jc, :],
                                 rhs=v_t[q][:],
                                 start=(q == 0), stop=False)
        for t in range(2 * NQ):
            q, vv = t % NQ, t // NQ
            for jc in range(NJC):
                nc.tensor.matmul(psj[jc][:],
                                 lhsT=mcatlf_sb[:, t, jc, :],
                                 rhs=v_lf[q][:, vv * BSH : (vv + 1) * BSH],
                                 start=False,
                                 stop=(t == 2 * NQ - 1))

        # ---- scale by per-sample c_b (columns), store transposed [OUT, BSH];
        # per-jc ops pipeline the scale against the out DMA ----
        out_sb = consts.tile([128, NJC, BSH], F32, tag="outsb")
        out_view = out_d[:].rearrange("(jc j) b -> j jc b", jc=NJC)
        for jc in range(NJC):
            nc.vector.tensor_mul(out_sb[:, jc, :], psj[jc][:], c_sb[:])
            if has_bias:
                nc.vector.tensor_scalar_add(out_sb[:, jc, :], out_sb[:, jc, :],
                                            bias_sb[:, jc : jc + 1])
        nc.scalar.dma_start(out=out_view[:, :, :], in_=out_sb[:])

    _split_sync_waits(nc)
    _CACHE[key] = nc
    return nc


def _quantize_noise_shaped(noise: np.ndarray) -> np.ndarray:
    """First-order error-feedback quantization to fp8 e3m4 along the time
    axis. noise: [B, NS, P] float32 -> [NS, B, P] e3m4 (time-major for the
    device DMA layout)."""
    q = np.empty((NS, B, P), dtype=F8_NP)
    carry = np.zeros((B, P), np.float32)
    for t in range(NS):
        v = noise[:, t, :] + carry
        qt = v.astype(F8_NP)
        q[t] = qt
        carry = v - qt.astype(np.float32)
    return q


def _host_precompute(decay_param, conv_w, conv_b, out_w, out_b):
    dp = float(np.asarray(decay_param).reshape(-1)[0])
    decay = 0.5 / (1.0 + np.exp(-dp))
    dt = 1.0 / NS
    retain = 1.0 - decay * dt

    s = np.arange(NS, dtype=np.float64)
    gA = (1.0 - retain ** (NS - s)) / (1.0 - retain)
    gL = retain ** (NS - 1 - s)

    # exact fp16 time-weights ride as the (tiny) moving operand of stage 1
    g3 = np.zeros((NS, 3), np.float64)
    g3[:, 0] = gA
    g3[:, 1] = gL
    g3[0, 2] = 1.0
    g3 = np.ascontiguousarray(g3.astype(F16_NP))

    conv_w = np.asarray(conv_w, np.float32)
    out_w = np.asarray(out_w, np.float32)
    w_sum = conv_w.sum(axis=2)
    t_sum = out_w @ w_sum              # [OUT, P]
    t0 = out_w @ conv_w[:, :, 0]
    t2 = out_w @ conv_w[:, :, 2]

    def tile_layout(r, np_dtype):
        # [K, OUT, P] -> [128, K*NQ, NJC, 128]: stationary tiles [128i, 128j]
        k = r.shape[0]
        njc = OUT // 128
        r_pad = np.zeros((k, OUT, NQ * 128), np.float32)
        r_pad[:, :, :P] = r
        m = r_pad.reshape(k, njc, 128, NQ, 128).transpose(4, 0, 3, 1, 2)
        return np.ascontiguousarray(
            m.reshape(128, k * NQ, njc, 128).astype(np_dtype))

    mcata = tile_layout(t_sum[None], F16_NP)           # fp16 A block
    # L rows x8 (V_L carries the 1/8), F rows x1: both ride fp8 e3m4
    mcatlf = tile_layout(np.stack([-8.0 * t0, -8.0 * t2]), F8_NP)

    bias_vec = (
        out_w @ np.asarray(conv_b, np.float32)
        + np.asarray(out_b, np.float32).reshape(OUT)
    )
    return g3, mcata, mcatlf, bias_vec


def kernel(x, noise, fb_w, fb_b, decay_param, conv_w, conv_b, out_w, out_b,
           _trace=False):
    global LAST_RUN

    x = np.asarray(x, np.float32)
    noise_q = _quantize_noise_shaped(np.asarray(noise, np.float32))  # [NS, B, P]

    g3, mcata, mcatlf, bias_vec = _host_precompute(
        decay_param, conv_w, conv_b, out_w, out_b)
    has_bias = bool(np.any(bias_vec != 0.0))

    # per-sample feedback scale: sigmoid(x . fb_w + fb_b) * sqrt(dt)/NS
    fb_w = np.asarray(fb_w, np.float32).reshape(IN)
    fb_b = float(np.asarray(fb_b, np.float32).reshape(-1)[0])
    z = x @ fb_w + fb_b
    cvec = (1.0 / (1.0 + np.exp(-z, dtype=np.float64))) * (np.sqrt(1.0 / NS) / NS)
    cvec = cvec.reshape(B).astype(np.float32)

    nc = _build_program(has_bias)

    in_maps = []
    for c in range(NCORES):
        sl = slice(c * BSH, (c + 1) * BSH)
        m = {
            "noise_sh": np.ascontiguousarray(noise_q[:, sl, :]),
            "g3": g3,
            "mcata": mcata,
            "mcatlf": mcatlf,
            "cvec": np.ascontiguousarray(cvec[sl].reshape(1, BSH)),
        }
        if has_bias:
            m["biasv"] = np.ascontiguousarray(bias_vec.reshape(OUT, 1))
        in_maps.append(m)

    res = run_bass_kernel_spmd(nc, in_maps, core_ids=list(range(NCORES)),
                               trace=_trace)
    LAST_RUN = res
    out = np.concatenate([m["out"].T for m in res.results], axis=0)
    return out.astype(np.float32)
